# revision 51
# baseline (speedup 1.0000x reference)
"""Trainium2 Bass kernel for nn_ArgmaxQuantize (vq_codebook).

Reference computation (per batch b):
    logits[n, p] = sum_c z[c, p] * proj_w[n, c] + proj_b[n]
    ind[p]       = argmax_n logits[n, p]          (first occurrence)
    z_q[:, p]    = embed[ind[p], :]               (softmax terms cancel exactly)
    diff         = zeros(1)

The straight-through output y_hard - sg(y_soft) + y_soft equals y_hard up to
~1e-7 relative, and argmax(softmax(x)) == argmax(x), so the kernel is
matmul -> argmax -> embedding-row gather.

Sharding: pure data-parallel over batch B=8 -> one batch per NeuronCore, no
collectives.  Per core:
  - matmul1 in a 3-term fp16 split (z = z1+z2, w = w1+w2; z1w1 + z1w2 + z2w1):
    fp32-grade accuracy (~2e-6 abs, zero argmax flips) at full PE speed.
  - pixels-on-partitions: PSUM tile [128 pix, 1024 codes].
  - ACT copies PSUM->SBUF; DVE MAX8 + FIND_INDEX8 give first-occurrence argmax.
  - per-tile indirect DMA ([128,1] row offsets) gathers embed rows into
    group tiles of 8 pixel-tiles; z_q leaves as pixel-major rows and the
    host transposes to [D, H, W].
"""

import numpy as np

import concourse.bass as bass
import concourse.tile as tile
from concourse import bacc, mybir
from concourse.bass_utils import run_bass_kernel_spmd

B, C, H, W = 8, 256, 64, 64
N, D = 1024, 256
NPIX = H * W            # 4096 pixels per batch/core
PT = NPIX // 128        # 32 pixel tiles
NK = 6                  # 3-term fp16 split -> 6 K-chunk matmuls of 128
NKZ = 4                 # z chunks stored: [z1(2), z2(2)] (z1 reused for 2 terms)
NKW = 4                 # w chunks stored: [w1(2), w2(2)]
ZMAP = [0, 1, 0, 1, 2, 3]   # z chunk used by matmul kc
WMAP = [0, 1, 2, 3, 0, 1]   # w chunk used by matmul kc
# z-DMA chunk sizes in pixel tiles (first chunks small so PE starts early)
ZCHUNKS = [2, 2, 4, 4, 4, 4, 4, 4, 4]
assert sum(ZCHUNKS) == PT
GROUPS = [8, 8, 8, 8]      # pixel tiles per gather group
assert sum(GROUPS) == PT
N_CORES = 8

_nc_cache = {}


def _build(with_bias: bool):
    f32 = mybir.dt.float32
    f16 = mybir.dt.float16
    u32 = mybir.dt.uint32
    i32 = mybir.dt.int32

    nc = bacc.Bacc("TRN2", target_bir_lowering=False, debug=False)
    # Z chunks: [partition, kc, pix-in-chunk] — per-partition contiguous runs
    Zs = [nc.declare_dram_parameter(f"Z{ch}", [128, NKZ, ln * 128], f16, isOutput=False)
          for ch, ln in enumerate(ZCHUNKS)]
    Wp = nc.declare_dram_parameter("Wp", [128, NKW, N], f16, isOutput=False)
    emb = nc.declare_dram_parameter("emb", [N, D], f32, isOutput=False)
    if with_bias:
        Bv = nc.declare_dram_parameter("Bv", [2, N], f16, isOutput=False)
    NG, GT = len(GROUPS), GROUPS[0]
    o_zq = nc.declare_dram_parameter("o_zq", [NG, 128, GT, D], f32, isOutput=True)
    o_ind = nc.declare_dram_parameter("o_ind", [NG, 128, GT], i32, isOutput=True)

    with tile.TileContext(nc) as tc:
        with (
            tc.tile_pool(name="w", bufs=1) as wpool,
            tc.tile_pool(name="z", bufs=1) as zpool,
            tc.tile_pool(name="lg", bufs=4) as lgpool,
            tc.tile_pool(name="mx", bufs=6) as mxpool,
            tc.tile_pool(name="ix", bufs=3) as ixpool,
            tc.tile_pool(name="zq", bufs=3) as zqpool,
            tc.tile_pool(name="ps", bufs=3, space="PSUM") as pspool,
        ):
            wk = wpool.tile([128, NKW, N], f16)
            nc.sync.dma_start(out=wk[:, 0:2, :], in_=Wp.ap()[:, 0:2, :])
            nc.sync.dma_start(out=wk[:, 2:4, :], in_=Wp.ap()[:, 2:4, :])
            if with_bias:
                bv = wpool.tile([2, N], f16)
                nc.sync.dma_start(out=bv[:, :], in_=Bv.ap()[:, :])
                ones2 = wpool.tile([2, 128], f16)
                nc.vector.memset(ones2[:, :], 1.0)

            # per-pixel-tile view of the chunked z tiles
            ztile = []
            for ch, ln in enumerate(ZCHUNKS):
                zt = zpool.tile([128, NKZ, ln * 128], f16, tag=f"zch{ch}")
                nc.sync.dma_start(out=zt[:, :, :], in_=Zs[ch].ap()[:, :, :])
                for j in range(ln):
                    ztile.append(zt[:, :, j * 128:(j + 1) * 128])

            t0 = 0
            for g, gt in enumerate(GROUPS):
                ixg = ixpool.tile([128, gt], i32, tag="ixg")
                zqg = zqpool.tile([128, gt, D], f32, tag="zqg")
                for j in range(gt):
                    t = t0 + j
                    ps = pspool.tile([128, N], f32, tag="ps")
                    zsl = ztile[t]
                    for kc in range(NK):
                        for cc in range(2):
                            nc.tensor.matmul(ps[:, cc * 512:(cc + 1) * 512],
                                             zsl[:, ZMAP[kc], :],
                                             wk[:, WMAP[kc], cc * 512:(cc + 1) * 512],
                                             start=(kc == 0),
                                             stop=(kc == NK - 1 and not with_bias))
                    if with_bias:
                        for cc in range(2):
                            nc.tensor.matmul(ps[:, cc * 512:(cc + 1) * 512],
                                             ones2[:, :],
                                             bv[:, cc * 512:(cc + 1) * 512],
                                             start=False, stop=True)
                    lg = lgpool.tile([128, N], f32, tag="lg")
                    nc.scalar.copy(lg[:, :], ps[:, :])
                    mv = mxpool.tile([128, 8], f32, tag="mv")
                    mi = mxpool.tile([128, 8], u32, tag="mi")
                    nc.vector.max(out=mv[:, :], in_=lg[:, :])
                    nc.vector.max_index(out=mi[:, :], in_max=mv[:, :],
                                        in_values=lg[:, :])
                    nc.vector.tensor_copy(ixg[:, j:j + 1], mi[:, 0:1].bitcast(i32))
                    nc.gpsimd.indirect_dma_start(
                        out=zqg[:, j, :],
                        out_offset=None,
                        in_=emb[:, :],
                        in_offset=bass.IndirectOffsetOnAxis(ap=mi[:, 0:1].bitcast(i32), axis=0),
                    )

                nc.scalar.dma_start(out=o_zq.ap()[g, :, :, :], in_=zqg[:, :, :])
                nc.scalar.dma_start(out=o_ind.ap()[g, :, :], in_=ixg[:, :])
                t0 += gt
    nc.compile()
    return nc


def _get_nc(with_bias: bool):
    if with_bias not in _nc_cache:
        _nc_cache[with_bias] = _build(with_bias)
    return _nc_cache[with_bias]


def _prepare_inputs(z, proj_w, proj_b, embed):
    z = np.ascontiguousarray(np.asarray(z, dtype=np.float32))
    proj_w = np.asarray(proj_w, dtype=np.float32)
    proj_b = np.asarray(proj_b, dtype=np.float32)
    embed = np.ascontiguousarray(np.asarray(embed, dtype=np.float32))

    wt = np.ascontiguousarray(proj_w.T)                      # [C, N]
    w1 = wt.astype(np.float16)
    w2 = (wt - w1.astype(np.float32)).astype(np.float16)
    W4 = np.concatenate([w1, w2], axis=0).reshape(NKW, 128, N)
    W4 = np.ascontiguousarray(W4.transpose(1, 0, 2))         # [128, NKW, N]

    with_bias = bool(np.any(proj_b))
    bias2 = None
    if with_bias:
        b1 = proj_b.astype(np.float16)
        b2 = (proj_b - b1.astype(np.float32)).astype(np.float16)
        bias2 = np.ascontiguousarray(np.stack([b1, b2], axis=0))  # [2, N]

    in_maps = []
    for b in range(B):
        zb = z[b].reshape(C, NPIX)                           # [256, 4096]
        z1 = zb.astype(np.float16)
        z2 = (zb - z1.astype(np.float32)).astype(np.float16)
        Z4 = np.concatenate([z1, z2], axis=0).reshape(NKZ, 128, NPIX)
        m = {"Wp": W4, "emb": embed}
        lo = 0
        for ch, ln in enumerate(ZCHUNKS):
            blk = Z4[:, :, lo * 128:(lo + ln) * 128].transpose(1, 0, 2)
            m[f"Z{ch}"] = np.ascontiguousarray(blk)
            lo += ln
        if with_bias:
            m["Bv"] = bias2
        in_maps.append(m)
    return in_maps, with_bias


def _run(z, proj_w, proj_b, embed, trace=False):
    in_maps, with_bias = _prepare_inputs(z, proj_w, proj_b, embed)
    nc = _get_nc(with_bias)
    res = run_bass_kernel_spmd(nc, in_maps, core_ids=list(range(N_CORES)), trace=trace)

    z_q = np.empty((B, D, H, W), dtype=np.float32)
    ind = np.empty((B, H, W), dtype=np.int32)
    for b in range(B):
        r = res.results[b]
        # o_zq[g, p, j, :] = z_q row of pixel (g*GT+j)*128 + p
        zq_rows = r["o_zq"].transpose(0, 2, 1, 3).reshape(NPIX, D)
        z_q[b] = zq_rows.T.reshape(D, H, W)
        ind[b] = r["o_ind"].transpose(0, 2, 1).reshape(NPIX).reshape(H, W)
    diff = np.zeros((1,), dtype=np.float32)
    return (z_q, diff, ind), res.exec_time_ns


def kernel(z, proj_w, proj_b, embed):
    out, _ = _run(z, proj_w, proj_b, embed, trace=False)
    return out


# revision 53
# speedup vs baseline: 1.2115x; 1.2115x over previous
"""Trainium2 Bass kernel for nn_ArgmaxQuantize (vq_codebook).

Reference computation (per batch b):
    logits[n, p] = sum_c z[c, p] * proj_w[n, c] + proj_b[n]
    ind[p]       = argmax_n logits[n, p]          (first occurrence)
    z_q[:, p]    = embed[ind[p], :]               (softmax terms cancel exactly)
    diff         = zeros(1)

The straight-through output y_hard - sg(y_soft) + y_soft equals y_hard up to
~1e-7 relative, and argmax(softmax(x)) == argmax(x), so the kernel is
matmul -> argmax -> embedding-row gather.

Sharding: pure data-parallel over batch B=8 -> one batch per NeuronCore, no
collectives.  Per core:
  - matmul1 in a 3-term fp16 split (z = z1+z2, w = w1+w2; z1w1 + z1w2 + z2w1):
    fp32-grade accuracy (~2e-6 abs, zero argmax flips) at full PE speed.
  - pixels-on-partitions: PSUM tile [128 pix, 1024 codes].
  - ACT copies PSUM->SBUF; DVE MAX8 + FIND_INDEX8 give first-occurrence argmax.
  - per-tile indirect DMA ([128,1] row offsets) gathers embed rows into
    group tiles of 8 pixel-tiles; z_q leaves as pixel-major rows and the
    host transposes to [D, H, W].
"""

import numpy as np

import concourse.bass as bass
import concourse.tile as tile
from concourse import bacc, mybir
from concourse.bass_utils import run_bass_kernel_spmd

B, C, H, W = 8, 256, 64, 64
N, D = 1024, 256
NPIX = H * W            # 4096 pixels per batch/core
PT = NPIX // 128        # 32 pixel tiles
NK = 6                  # 3-term fp16 split -> 6 K-chunk matmuls of 128
NKZ = 4                 # z chunks stored: [z1(2), z2(2)] (z1 reused for 2 terms)
NKW = 4                 # w chunks stored: [w1(2), w2(2)]
ZMAP = [0, 1, 0, 1, 2, 3]   # z chunk used by matmul kc
WMAP = [0, 1, 2, 3, 0, 1]   # w chunk used by matmul kc
# z-DMA chunk sizes in pixel tiles (first chunks small so PE starts early)
ZCHUNKS = [2, 2, 4, 4, 4, 4, 4, 4, 4]
assert sum(ZCHUNKS) == PT
GROUPS = [8, 8, 8, 8]      # pixel tiles per gather group
assert sum(GROUPS) == PT
N_CORES = 8

_nc_cache = {}


def _build(with_bias: bool):
    f32 = mybir.dt.float32
    f16 = mybir.dt.float16
    u32 = mybir.dt.uint32
    i32 = mybir.dt.int32

    nc = bacc.Bacc("TRN2", target_bir_lowering=False, debug=False)
    # Z chunks: [partition, kc, pix-in-chunk] — per-partition contiguous runs
    Zs = [nc.declare_dram_parameter(f"Z{ch}", [128, NKZ, ln * 128], f16, isOutput=False)
          for ch, ln in enumerate(ZCHUNKS)]
    Wp = nc.declare_dram_parameter("Wp", [128, NKW, N], f16, isOutput=False)
    emb = nc.declare_dram_parameter("emb", [N, D], f32, isOutput=False)
    if with_bias:
        Bv = nc.declare_dram_parameter("Bv", [2, N], f16, isOutput=False)
    NG, GT = len(GROUPS), GROUPS[0]
    o_zq = nc.declare_dram_parameter("o_zq", [NG, 128, GT, D], f32, isOutput=True)
    o_ind = nc.declare_dram_parameter("o_ind", [NG, 128, GT], i32, isOutput=True)

    with tile.TileContext(nc) as tc:
        with (
            tc.tile_pool(name="w", bufs=1) as wpool,
            tc.tile_pool(name="z", bufs=1) as zpool,
            tc.tile_pool(name="lg", bufs=4) as lgpool,
            tc.tile_pool(name="mx", bufs=6) as mxpool,
            tc.tile_pool(name="ix", bufs=3) as ixpool,
            tc.tile_pool(name="zq", bufs=3) as zqpool,
            tc.tile_pool(name="ps", bufs=3, space="PSUM") as pspool,
        ):
            wk = wpool.tile([128, NKW, N], f16)
            nc.sync.dma_start(out=wk[:, 0:2, :], in_=Wp.ap()[:, 0:2, :])
            if with_bias:
                bv = wpool.tile([2, N], f16)
                nc.sync.dma_start(out=bv[:, :], in_=Bv.ap()[:, :])
                ones2 = wpool.tile([2, 128], f16)
                nc.vector.memset(ones2[:, :], 1.0)

            # per-pixel-tile view of the chunked z tiles; the first matmul needs
            # only wk[0:2] + z chunk 0, so wk[2:4] is issued after z chunk 0
            ztile = []
            for ch, ln in enumerate(ZCHUNKS):
                zt = zpool.tile([128, NKZ, ln * 128], f16, tag=f"zch{ch}")
                nc.sync.dma_start(out=zt[:, :, :], in_=Zs[ch].ap()[:, :, :])
                if ch == 0:
                    nc.sync.dma_start(out=wk[:, 2:4, :], in_=Wp.ap()[:, 2:4, :])
                for j in range(ln):
                    ztile.append(zt[:, :, j * 128:(j + 1) * 128])

            t0 = 0
            for g, gt in enumerate(GROUPS):
                ixg = ixpool.tile([128, gt], i32, tag="ixg")
                zqg = zqpool.tile([128, gt, D], f32, tag="zqg")
                for j in range(gt):
                    t = t0 + j
                    ps = pspool.tile([128, N], f32, tag="ps")
                    zsl = ztile[t]
                    for kc in range(NK):
                        for cc in range(2):
                            nc.tensor.matmul(ps[:, cc * 512:(cc + 1) * 512],
                                             zsl[:, ZMAP[kc], :],
                                             wk[:, WMAP[kc], cc * 512:(cc + 1) * 512],
                                             start=(kc == 0),
                                             stop=(kc == NK - 1 and not with_bias))
                    if with_bias:
                        for cc in range(2):
                            nc.tensor.matmul(ps[:, cc * 512:(cc + 1) * 512],
                                             ones2[:, :],
                                             bv[:, cc * 512:(cc + 1) * 512],
                                             start=False, stop=True)
                    lg = lgpool.tile([128, N], f32, tag="lg")
                    nc.scalar.copy(lg[:, :], ps[:, :])
                    mv = mxpool.tile([128, 8], f32, tag="mv")
                    mi = mxpool.tile([128, 8], u32, tag="mi")
                    nc.vector.max(out=mv[:, :], in_=lg[:, :])
                    nc.vector.max_index(out=mi[:, :], in_max=mv[:, :],
                                        in_values=lg[:, :])
                    nc.vector.tensor_copy(ixg[:, j:j + 1], mi[:, 0:1].bitcast(i32))
                    nc.gpsimd.indirect_dma_start(
                        out=zqg[:, j, :],
                        out_offset=None,
                        in_=emb[:, :],
                        in_offset=bass.IndirectOffsetOnAxis(ap=mi[:, 0:1].bitcast(i32), axis=0),
                    )

                nc.scalar.dma_start(out=o_zq.ap()[g, :, :, :], in_=zqg[:, :, :])
                nc.scalar.dma_start(out=o_ind.ap()[g, :, :], in_=ixg[:, :])
                t0 += gt
    nc.compile()
    return nc


def _get_nc(with_bias: bool):
    if with_bias not in _nc_cache:
        _nc_cache[with_bias] = _build(with_bias)
    return _nc_cache[with_bias]


def _prepare_inputs(z, proj_w, proj_b, embed):
    z = np.ascontiguousarray(np.asarray(z, dtype=np.float32))
    proj_w = np.asarray(proj_w, dtype=np.float32)
    proj_b = np.asarray(proj_b, dtype=np.float32)
    embed = np.ascontiguousarray(np.asarray(embed, dtype=np.float32))

    wt = np.ascontiguousarray(proj_w.T)                      # [C, N]
    w1 = wt.astype(np.float16)
    w2 = (wt - w1.astype(np.float32)).astype(np.float16)
    W4 = np.concatenate([w1, w2], axis=0).reshape(NKW, 128, N)
    W4 = np.ascontiguousarray(W4.transpose(1, 0, 2))         # [128, NKW, N]

    with_bias = bool(np.any(proj_b))
    bias2 = None
    if with_bias:
        b1 = proj_b.astype(np.float16)
        b2 = (proj_b - b1.astype(np.float32)).astype(np.float16)
        bias2 = np.ascontiguousarray(np.stack([b1, b2], axis=0))  # [2, N]

    in_maps = []
    for b in range(B):
        zb = z[b].reshape(C, NPIX)                           # [256, 4096]
        z1 = zb.astype(np.float16)
        z2 = (zb - z1.astype(np.float32)).astype(np.float16)
        Z4 = np.concatenate([z1, z2], axis=0).reshape(NKZ, 128, NPIX)
        m = {"Wp": W4, "emb": embed}
        lo = 0
        for ch, ln in enumerate(ZCHUNKS):
            blk = Z4[:, :, lo * 128:(lo + ln) * 128].transpose(1, 0, 2)
            m[f"Z{ch}"] = np.ascontiguousarray(blk)
            lo += ln
        if with_bias:
            m["Bv"] = bias2
        in_maps.append(m)
    return in_maps, with_bias


def _run(z, proj_w, proj_b, embed, trace=False):
    in_maps, with_bias = _prepare_inputs(z, proj_w, proj_b, embed)
    nc = _get_nc(with_bias)
    res = run_bass_kernel_spmd(nc, in_maps, core_ids=list(range(N_CORES)), trace=trace)

    z_q = np.empty((B, D, H, W), dtype=np.float32)
    ind = np.empty((B, H, W), dtype=np.int32)
    for b in range(B):
        r = res.results[b]
        # o_zq[g, p, j, :] = z_q row of pixel (g*GT+j)*128 + p
        zq_rows = r["o_zq"].transpose(0, 2, 1, 3).reshape(NPIX, D)
        z_q[b] = zq_rows.T.reshape(D, H, W)
        ind[b] = r["o_ind"].transpose(0, 2, 1).reshape(NPIX).reshape(H, W)
    diff = np.zeros((1,), dtype=np.float32)
    return (z_q, diff, ind), res.exec_time_ns


def kernel(z, proj_w, proj_b, embed):
    out, _ = _run(z, proj_w, proj_b, embed, trace=False)
    return out
